# revision 1
# baseline (speedup 1.0000x reference)
"""ALiBi multi-head attention with LoRA projections on 8 TRN2 NeuronCores.

Sharding: query-parallel. Core c handles batch b=c//4, query rows
[512*(c%4), 512*(c%4+1)) of that batch, all 16 heads.  K/V are computed
for the local 512 tokens and AllGathered (bf16) within each 4-core batch
group.  The non-causal ALiBi softmax factorizes as
    softmax(s_ij + slope*(j-i))_j = exp(s_ij) * E_j / sum_j exp(s_ij) * E_j,
      E_j = exp(slope*(j - (S-1)))
so no row-max / row-sum passes are needed: E is folded into V (and an
extra all-ones column of V yields the denominator as matmul output).
Heads with large slopes only attend to the last few key tiles
(contributions beyond that are < ~e^-19 relative) -> per-head key ranges.

The attention loop is software-pipelined: scores+exp run LOOKAHEAD units
ahead of the PV matmuls, so ScalarE exp work fills the AllGather-V window.
"""

import os
import sys
import threading

import numpy as np
import ml_dtypes

sys.path.insert(0, "/opt/trn_rl_repo")

B, S, E, H, D, R = 2, 2048, 1024, 16, 64, 8
NC = 8
TQ = S // 4          # 512 tokens per core
# key tiles (of 128) per head, ranges end at S; ~22*2^h keys needed
KT = [1, 1, 1, 2, 3, 6, 12] + [16] * 9
# per-dp K-load tile count (rank-aligned roundup of KT[2dp+1])
TL = [1, 2, 8, 16, 16, 16, 16, 16]
F32 = np.float32
BF16 = ml_dtypes.bfloat16
LOOKAHEAD = 14

# per-kt V'' load groups: (kt0, nkt, hmin)
VG_GROUPS = [(0, 4, 7), (4, 6, 6), (10, 3, 5), (13, 1, 4), (14, 1, 3), (15, 1, 0)]


def _kt_group(kt):
    for gi, (kt0, nkt, hmin) in enumerate(VG_GROUPS):
        if kt0 <= kt < kt0 + nkt:
            return gi, kt - kt0, hmin
    raise AssertionError(kt)


_BUILT = None
_LOCK = threading.Lock()


def _build():
    import concourse.bass as bass
    import concourse.tile as tile
    from concourse import bacc, mybir

    f32 = mybir.dt.float32
    bf16 = mybir.dt.bfloat16
    AF = mybir.ActivationFunctionType
    ALU = mybir.AluOpType

    nc = bacc.Bacc(
        "TRN2", target_bir_lowering=False, debug=False,
        enable_asserts=False, num_devices=NC,
    )

    def din(name, shape, dt):
        return nc.dram_tensor(name, shape, dt, kind="ExternalInput").ap()

    xT = din("xT", [E, TQ], f32)
    xTb = din("xTb", [E, TQ], bf16)
    WT = {n: din(f"W{n}T", [E, E], bf16) for n in "qkvo"}
    AT = din("AT", [E, 27], bf16)
    AoT = din("AoT", [E, R], bf16)
    BALL = din("BALL", [128, E], bf16)
    BoT = din("BoT", [9, E], bf16)
    ETd = din("ET", [TQ, H], f32)
    rzd = din("rz", [128, 1], f32)
    out_d = nc.dram_tensor("out", [E, TQ], f32, kind="ExternalOutput").ap()

    with tile.TileContext(nc) as tc:
        import contextlib
        ctx = contextlib.ExitStack()
        dram = ctx.enter_context(tc.tile_pool(name="dram", bufs=1, space="DRAM"))
        kin = dram.tile([E, TQ], bf16)
        kg = dram.tile([4 * E, TQ], bf16)
        vin = dram.tile([TQ, H * 65], bf16)
        vg = dram.tile([S, H * 65], bf16)

        cpool = ctx.enter_context(tc.tile_pool(name="consts", bufs=1))
        wpool = ctx.enter_context(tc.tile_pool(name="work", bufs=1))
        ppool = ctx.enter_context(tc.tile_pool(name="ptiles", bufs=LOOKAHEAD + 2))
        spool = ctx.enter_context(tc.tile_pool(name="small", bufs=2))
        # one PSUM pool, 8 banks: tag "big" ([128,1024] f32 = 2 banks, 2 bufs)
        # shared by proj/t1/scores/bcast; tag "ot" ([65,512] = 1 bank, 4 bufs)
        # so two head-pairs' PV accumulators can coexist (the eviction chain
        # no longer blocks the next pair's PV start).
        psum = ctx.enter_context(tc.tile_pool(name="psum", bufs=2, space="PSUM"))

        # ---- critical-path loads on the sync queue; bulk consts go on the
        # scalar-engine HWDGE queue so they don't block the K path ----
        x_b = wpool.tile([128, 8, TQ], bf16, name="x_b", tag="xb_ot")
        nc.sync.dma_start(x_b[:], xTb.rearrange("(k p) t -> p k t", p=128))
        AT_sb = cpool.tile([128, 8, 27], bf16, name="AT_sb")
        nc.sync.dma_start(AT_sb[:], AT.rearrange("(k p) m -> p k m", p=128))
        Ball_sb = cpool.tile([128, E], bf16, name="Ball_sb")
        nc.sync.dma_start(Ball_sb[:], BALL[:, :])
        W_sb = {}
        W_sb["k"] = wpool.tile([128, 8, E], bf16, name="Wk_sb", tag="wk_wo")
        nc.sync.dma_start(W_sb["k"][:], WT["k"].rearrange("(k p) m -> p k m", p=128))
        for n in "vq":
            W_sb[n] = wpool.tile([128, 8, E], bf16, name=f"W{n}_sb", tag=f"w{n}")
            nc.scalar.dma_start(W_sb[n][:], WT[n].rearrange("(k p) m -> p k m", p=128))
        ET_sb = cpool.tile([128, 4, H], f32, name="ET_sb")
        nc.scalar.dma_start(ET_sb[:], ETd.rearrange("(tt p) h -> p tt h", p=128))

        ones1 = cpool.tile([1, 64], bf16, name="ones1")
        nc.vector.memset(ones1[:], 1.0)
        ones512 = cpool.tile([1, TQ], bf16, name="ones512")
        nc.vector.memset(ones512[:], 1.0)
        e8 = cpool.tile([1, 9], bf16, name="e8")
        nc.vector.memset(e8[:], 0.0)
        nc.vector.memset(e8[:, 8:9], 1.0)

        # warm the ACT exp table early (table load ~2.7us overlaps with DMAs)
        warm = cpool.tile([1, 16], f32, name="warm")
        nc.vector.memset(warm[:], 0.0)
        nc.scalar.activation(warm[:], warm[:], AF.Exp)


        # ---- t1 = lora-A down-proj for q,k,v; row groups at bases 0/32/64
        # with a trailing all-ones row each (via e8 x ones matmul) ----
        ps_t1 = psum.tile([73, TQ], f32, tag="big", name="ps_t1")
        for gi, c0 in ((1, 9), (0, 0), (2, 18)):   # k group first
            nc.tensor.matmul(ps_t1[32 * gi:32 * gi + 9, :], e8[:],
                             ones512[:], start=True, stop=False)
            for k in range(8):
                nc.tensor.matmul(ps_t1[32 * gi:32 * gi + 8, :],
                                 AT_sb[:, k, c0:c0 + 8], x_b[:, k, :],
                                 start=False, stop=(k == 7))
        t1 = wpool.tile([128, TQ], bf16, name="t1")
        for gi in range(3):
            nc.vector.tensor_copy(t1[32 * gi:32 * gi + 9, :],
                                  ps_t1[32 * gi:32 * gi + 9, :])

        # ---- K projection (transposed layout [d, tok]) + AllGather ----
        Kloc = wpool.tile([128, 8, TQ], bf16, name="Kloc", tag="kq")
        for m in range(8):
            ps = psum.tile([128, TQ], f32, tag="big", name="ps_proj")
            for k in range(8):
                nc.tensor.matmul(ps[:], W_sb["k"][:, k, m * 128:(m + 1) * 128],
                                 x_b[:, k, :], start=(k == 0), stop=False)
            nc.tensor.matmul(ps[:], Ball_sb[32:41, m * 128:(m + 1) * 128],
                             t1[32:41, :], start=False, stop=True)
            nc.vector.tensor_copy(Kloc[:, m, :], ps[:])
        nc.sync.dma_start(kin.rearrange("(m p) t -> p m t", p=128), Kloc[:])
        nc.gpsimd.collective_compute(
            "AllGather", mybir.AluOpType.bypass,
            replica_groups=[[0, 1, 2, 3], [4, 5, 6, 7]],
            ins=[kin.opt()], outs=[kg.opt()],
        )

        # ---- V projection (natural layout [tok, d]), E-scaled, + E columns ----
        V2 = wpool.tile([128, 4, H * 65], bf16, name="V2", tag="v2")
        for tt in range(4):
            for nh in range(2):
                ps = psum.tile([128, 512], f32, tag="big", name="ps_projv")
                for k in range(8):
                    nc.tensor.matmul(ps[:], x_b[:, k, tt * 128:(tt + 1) * 128],
                                     W_sb["v"][:, k, nh * 512:(nh + 1) * 512],
                                     start=(k == 0), stop=False)
                nc.tensor.matmul(ps[:], t1[64:73, tt * 128:(tt + 1) * 128],
                                 Ball_sb[64:73, nh * 512:(nh + 1) * 512],
                                 start=False, stop=True)
                outv = V2[:, tt, nh * 520:nh * 520 + 520]
                outv = outv.rearrange("p (n d) -> p n d", d=65)[:, :, 0:64]
                inv = ps[:].rearrange("p (n d) -> p n d", d=64)
                eap = ET_sb[:, tt, nh * 8:(nh + 1) * 8]
                ebc = bass.AP(eap.tensor, eap.offset,
                              [list(eap.ap[0]), list(eap.ap[1]), [0, 64]])
                nc.vector.tensor_tensor(outv, inv, ebc, op=ALU.mult)
            nc.vector.tensor_copy(V2[:, tt, 64:H * 65:65], ET_sb[:, tt, :])
        nc.sync.dma_start(vin.rearrange("(tt p) c -> p tt c", p=128), V2[:])
        nc.gpsimd.collective_compute(
            "AllGather", mybir.AluOpType.bypass,
            replica_groups=[[0, 1, 2, 3], [4, 5, 6, 7]],
            ins=[vin.opt()], outs=[vg.opt()],
        )

        # ---- Q projection (transposed layout [d, q]); evictions on ScalarE
        # (idle until the attention exp stream begins) ----
        Q_sb = wpool.tile([128, 8, TQ], bf16, name="Q_sb", tag="kq")
        for m in range(8):
            ps = psum.tile([128, TQ], f32, tag="big", name="ps_proj")
            for k in range(8):
                nc.tensor.matmul(ps[:], W_sb["q"][:, k, m * 128:(m + 1) * 128],
                                 x_b[:, k, :], start=(k == 0), stop=False)
            nc.tensor.matmul(ps[:], Ball_sb[0:9, m * 128:(m + 1) * 128],
                             t1[0:9, :], start=False, stop=True)
            nc.scalar.copy(Q_sb[:, m, :], ps[:])

        # ---- load gathered K (per d-pair, rank-aligned key ranges) ----
        kgv = kg.rearrange("(r d p) t -> p r d t", d=8, p=128)
        Ksb = []
        for dp in range(8):
            T = TL[dp]
            t = cpool.tile([128, T * 128], bf16, name=f"Ksb{dp}")
            if T >= 4:
                nr = T // 4
                src = kgv[:, 4 - nr:4, dp, :]
                dst = t.rearrange("p (r t) -> p r t", t=512)
            else:
                tw = T * 128
                src = kgv[:, 3, dp, 512 - tw:512]
                dst = t[:]
            nc.sync.dma_start(dst, src)
            Ksb.append(t)

        # ---- load gathered V'' (per kt group, needed head tail only);
        # late-kt groups first: the early attention units need them ----
        vgv = vg.rearrange("(kt p) c -> p kt c", p=128)
        Vg = [None] * len(VG_GROUPS)
        for gi in reversed(range(len(VG_GROUPS))):
            kt0, nkt, hmin = VG_GROUPS[gi]
            c0 = 65 * hmin
            t = cpool.tile([128, nkt, H * 65 - c0], bf16, name=f"Vg{kt0}")
            nc.sync.dma_start(t[:], vgv[:, kt0:kt0 + nkt, c0:])
            Vg[gi] = t

        def v2slice(kt, h):
            gi, ki, hmin = _kt_group(kt)
            c = (h - hmin) * 65
            return Vg[gi][:, ki, c:c + 65]

        # ---- attention, software-pipelined: scores+exp LOOKAHEAD units
        # ahead of PV so exp fills the AllGather-V window ----
        OT = wpool.tile([128, 8, TQ], bf16, name="OT", tag="xb_ot")
        units = []
        for hp in range(8):
            T1 = KT[2 * hp + 1]
            for kt in range(16 - T1, 16):
                units.append((hp, kt))
        nU = len(units)
        Pt = {}
        psO = {}
        first = {}
        for step in range(nU + LOOKAHEAD):
            if step < nU:
                hp, kt = units[step]
                T0 = KT[2 * hp]
                paired = kt >= 16 - T0
                koff = kt - (16 - TL[hp])
                ps = psum.tile([128, 1024], f32, tag="big", name=f"psS{step}")
                nc.tensor.matmul(ps[:, 0:512],
                                 Ksb[hp][64:128, koff * 128:(koff + 1) * 128],
                                 Q_sb[64:128, hp, :], start=True, stop=True)
                if paired:
                    nc.tensor.matmul(ps[:, 512:1024],
                                     Ksb[hp][0:64, koff * 128:(koff + 1) * 128],
                                     Q_sb[0:64, hp, :], start=True, stop=True)
                P = ppool.tile([128, 1024], bf16, tag="p", name=f"P{step}")
                if paired:
                    nc.scalar.activation(P[:], ps[:], AF.Exp)
                else:
                    nc.scalar.activation(P[:, 0:512], ps[:, 0:512], AF.Exp)
                Pt[step] = P
            j = step - LOOKAHEAD
            if j < 0:
                continue
            hp, kt = units[j]
            T0 = KT[2 * hp]
            paired = kt >= 16 - T0
            if hp not in psO:
                psO[hp] = [psum.tile([65, TQ], f32, tag="ot", bufs=4,
                                     name=f"psO{hp}_{i}") for i in range(2)]
                first[hp] = [True, True]
            P = Pt.pop(j)
            nc.tensor.matmul(psO[hp][1][:], v2slice(kt, 2 * hp + 1), P[:, 0:512],
                             start=first[hp][1], stop=(kt == 15))
            first[hp][1] = False
            if paired:
                nc.tensor.matmul(psO[hp][0][:], v2slice(kt, 2 * hp), P[:, 512:1024],
                                 start=first[hp][0], stop=(kt == 15))
                first[hp][0] = False
            if kt == 15:
                for i in range(2):
                    lsb = spool.tile([1, TQ], f32, tag="lsb", bufs=2, name=f"l{hp}_{i}")
                    nc.vector.tensor_copy(lsb[:], psO[hp][i][64:65, :])
                    recf = spool.tile([1, TQ], f32, tag="recf", bufs=2, name=f"rf{hp}_{i}")
                    nc.vector.reciprocal_approx_fast(recf[:], lsb[:])
                    rec = spool.tile([1, TQ], bf16, tag="rec", bufs=2, name=f"rec{hp}_{i}")
                    nc.vector.tensor_copy(rec[:], recf[:])
                    onum = spool.tile([64, TQ], bf16, tag="onum", bufs=2, name=f"on{hp}_{i}")
                    nc.vector.tensor_copy(onum[:], psO[hp][i][0:64, :])
                    bc = psum.tile([64, TQ], f32, tag="big", name=f"bc{hp}_{i}")
                    nc.tensor.matmul(bc[:], ones1[:], rec[:], start=True, stop=True)
                    nc.vector.tensor_mul(OT[64 * i:64 * i + 64, hp, :], onum[:], bc[:])
                del psO[hp]

        # ---- late consts for the O path (Wo reuses Wk's SBUF slot) ----
        W_sb["o"] = wpool.tile([128, 8, E], bf16, name="Wo_sb", tag="wk_wo")
        nc.scalar.dma_start(W_sb["o"][:], WT["o"].rearrange("(k p) m -> p k m", p=128))
        AoT_sb = cpool.tile([128, 8, R], bf16, name="AoT_sb")
        nc.scalar.dma_start(AoT_sb[:], AoT.rearrange("(k p) m -> p k m", p=128))
        Bo_sb = cpool.tile([9, E], bf16, name="Bo_sb")
        nc.scalar.dma_start(Bo_sb[:], BoT[:, :])
        rz_sb = cpool.tile([128, 1], f32, name="rz_sb")
        nc.scalar.dma_start(rz_sb[:], rzd[:, :])
        x_f2 = wpool.tile([128, 8, TQ], f32, name="x_f2", tag="xf")
        nc.scalar.dma_start(x_f2[:], xT.rearrange("(k p) t -> p k t", p=128))

        # ---- lora-o down-proj ----
        ps_t2 = psum.tile([9, TQ], f32, tag="big", name="ps_t2")
        nc.tensor.matmul(ps_t2[:], e8[:], ones512[:], start=True, stop=False)
        for k in range(8):
            nc.tensor.matmul(ps_t2[0:8, :], AoT_sb[:, k, :], OT[:, k, :],
                             start=False, stop=(k == 7))
        t2 = wpool.tile([9, TQ], bf16, name="t2")
        nc.vector.tensor_copy(t2[:], ps_t2[:])

        # ---- O projection + rezero residual ----
        out_sb = wpool.tile([128, 8, TQ], f32, name="out_sb", tag="v2")
        for m in range(8):
            ps = psum.tile([128, TQ], f32, tag="big", name="ps_proj")
            for k in range(8):
                nc.tensor.matmul(ps[:], W_sb["o"][:, k, m * 128:(m + 1) * 128],
                                 OT[:, k, :], start=(k == 0), stop=False)
            nc.tensor.matmul(ps[:], Bo_sb[:, m * 128:(m + 1) * 128],
                             t2[:], start=False, stop=True)
            nc.vector.scalar_tensor_tensor(
                out_sb[:, m, :], ps[:], rz_sb[:, 0:1], x_f2[:, m, :],
                op0=ALU.mult, op1=ALU.add)
        odv = out_d.rearrange("(m p) t -> p m t", p=128)
        nc.sync.dma_start(odv[:, 0:4, :], out_sb[:, 0:4, :])
        nc.sync.dma_start(odv[:, 4:8, :], out_sb[:, 4:8, :])
        ctx.close()

    if not os.environ.get("BASS_SKIP_COMPILE"):
        nc.compile()
    return nc


def _get_built():
    global _BUILT
    with _LOCK:
        if _BUILT is None:
            _BUILT = _build()
    return _BUILT


def _prep_inputs(inputs):
    """Host-side sharding + weight relayout. Returns in_maps for 8 cores."""
    x = np.asarray(inputs["x"], F32)
    rez = float(np.asarray(inputs["rezero"]).reshape(-1)[0])

    def bf(a):
        return np.ascontiguousarray(a.astype(BF16))

    WqT = bf(np.asarray(inputs["Wq"], F32).T / 8.0)
    WkT = bf(np.asarray(inputs["Wk"], F32).T)
    WvT = bf(np.asarray(inputs["Wv"], F32).T)
    WoT = bf(np.asarray(inputs["Wo"], F32).T)
    AT = np.zeros((E, 27), F32)
    AT[:, 0:8] = np.asarray(inputs["Aq"], F32).T
    AT[:, 9:17] = np.asarray(inputs["Ak"], F32).T
    AT[:, 18:26] = np.asarray(inputs["Av"], F32).T
    AT = bf(AT)
    AoT = bf(np.asarray(inputs["Ao"], F32).T)

    def baug(Bm, bias, scale):
        out = np.zeros((9, E), F32)
        out[0:8] = np.asarray(Bm, F32).T * scale
        out[8] = np.asarray(bias, F32) * (scale * 8.0)
        return out

    BALLf = np.zeros((128, E), F32)
    BALLf[0:9] = baug(inputs["Bq"], inputs["bq"], 1.0 / 64.0)
    BALLf[32:41] = baug(inputs["Bk"], inputs["bk"], 1.0 / 8.0)
    BALLf[64:73] = baug(inputs["Bv"], inputs["bv"], 1.0 / 8.0)
    BALL = bf(BALLf)
    BoT = bf(baug(inputs["Bo"], inputs["bo"], 1.0 / 8.0))

    slopes = 0.5 ** np.arange(H, dtype=F32)
    jpos = np.arange(S, dtype=F32)
    Efull = np.exp(slopes[:, None] * (jpos[None, :] - (S - 1))).astype(F32)  # [H,S]
    rz_vec = np.full((128, 1), rez, F32)

    in_maps = []
    for c in range(NC):
        b, r = c // 4, c % 4
        sl = slice(TQ * r, TQ * (r + 1))
        m = {
            "xT": np.ascontiguousarray(x[b, sl, :].T),
            "xTb": bf(x[b, sl, :].T),
            "WqT": WqT, "WkT": WkT, "WvT": WvT, "WoT": WoT,
            "AT": AT, "AoT": AoT,
            "BALL": BALL, "BoT": BoT,
            "ET": np.ascontiguousarray(Efull[:, sl].T),
            "rz": rz_vec,
        }
        in_maps.append(m)
    return in_maps


def kernel(**inputs) -> np.ndarray:
    from concourse import bass_utils

    nc = _get_built()
    in_maps = _prep_inputs(inputs)
    res = bass_utils.run_bass_kernel_spmd(nc, in_maps, core_ids=list(range(NC)))
    out = np.zeros((B, S, E), F32)
    for c in range(NC):
        b, r = c // 4, c % 4
        out[b, TQ * r:TQ * (r + 1), :] = res.results[c]["out"].T
    return out


if __name__ == "__main__":
    _get_built()
    print("build+compile OK")



# revision 10
# speedup vs baseline: 1.2904x; 1.2904x over previous
"""ALiBi multi-head attention with LoRA projections on 8 TRN2 NeuronCores.

Sharding: query-parallel. Core c handles batch b=c//4, query rows
[512*(c%4), 512*(c%4+1)) of that batch, all 16 heads.  K/V are computed
for the local 512 tokens and AllGathered within each 4-core batch group.

The non-causal ALiBi softmax factorizes as
    softmax(s_ij + slope*(j-i))_j = exp(s_ij) * E_j / sum_j exp(s_ij) * E_j,
      E_j = exp(slope*(j - (S-1)))
E is folded into V (and an extra E column of V yields the denominator as
matmul output), so no row-max / row-sum passes are needed.

fp8 (e4m3, TRN max 240) everywhere on the matmul path:
  - x, W (x32 host-scaled), LoRA A (x32) / B, Q', K', V''=32*V*E, P=exp(s),
    attention out OT = 32*attnout.  Scale compensations are powers of two
    folded into host weights, one activation scale (2^-13/8) and the final
    rezero/1024 residual scale.
  - Projections and PV matmuls use MatmulPerfMode.DoubleRow (256-wide
    contraction at 0.5 cyc/row): 2-4x faster than bf16.
  - Both AllGathers carry fp8 (0.5 MB < 1 MB) -> Mesh algorithm.
  - fp8's subnormal flush (~2^-9) on V''*E implements the per-head key-range
    truncation; KT trims the computed ranges to what survives (131 of 256
    key tiles).  Verified numerically: rel_err 8.2e-4 vs 2e-2 tolerance.

All DRAM tensors are host-pre-blocked to [128, X] contiguous layouts so
every DMA is descriptor-friendly (the baseline's strided rearrange loads
cost 20+us each).

The attention loop is software-pipelined: scores+exp run LOOKAHEAD groups
ahead of the PV matmuls so ScalarE exp (the bottleneck, ~70us) covers the
AllGather-V window.
"""

import os
import sys
import threading

import numpy as np
import ml_dtypes

sys.path.insert(0, "/opt/trn_rl_repo")

B, S, E, H, D, R = 2, 2048, 1024, 16, 64, 8
NC = 8
TQ = S // 4          # 512 tokens per core
F32 = np.float32
F8 = ml_dtypes.float8_e4m3

# key tiles (of 128) per head; ranges end at S.  fp8 flush of V''*E zeroes
# contributions below ~e^-9 relative, so anything beyond these is noise.
KT = [1, 1, 1, 1, 1, 2, 3, 4, 8, 13, 16, 16, 16, 16, 16, 16]
# per-head-pair K-load tile count (covers the odd head's range; multiples
# of 4 when >4 so the load maps to whole source ranks)
TL = [1, 1, 2, 4, 16, 16, 16, 16]
# V'' load groups: (kt0, nkt=4, hmin).  4-aligned so tile-pairs never span.
VG_GROUPS = [(0, 4, 9), (4, 4, 9), (8, 4, 8), (12, 4, 0)]

LOOKAHEAD = 40       # exp groups ahead of PV (covers the AllGather-V window)


def _exp_groups():
    """[(h, kt0, paired, start, stop)] in program order."""
    out = []
    for h in range(H):
        T = KT[h]
        first = 16 - T
        kts = list(range(first, 16))
        items = []
        if T % 2 == 1:
            items.append((kts[0], False))
            kts = kts[1:]
        for i in range(0, len(kts), 2):
            items.append((kts[i], True))
        for idx, (kt0, paired) in enumerate(items):
            start = idx == 0
            stop = idx == len(items) - 1
            out.append((h, kt0, paired, start, stop))
    return out


GROUPS = _exp_groups()

_BUILT = None
_LOCK = threading.Lock()


def _build():
    import concourse.bass as bass
    import concourse.tile as tile
    from concourse import bacc, mybir

    f32 = mybir.dt.float32
    bf16 = mybir.dt.bfloat16
    fp8 = mybir.dt.float8e4
    AF = mybir.ActivationFunctionType
    ALU = mybir.AluOpType
    DR = mybir.MatmulPerfMode.DoubleRow

    nc = bacc.Bacc(
        "TRN2", target_bir_lowering=False, debug=False,
        enable_asserts=False, num_devices=NC,
    )

    def din(name, shape, dt):
        return nc.dram_tensor(name, shape, dt, kind="ExternalInput").ap()

    x8d = din("x8", [128, 8 * TQ], fp8)          # fp8 x, blocked [p, k, t]
    xfd = din("xf", [128, 8 * TQ], f32)          # f32 x for the residual
    Wd = {n: din(f"W{n}", [128, 8 * E], fp8) for n in "qkvo"}  # 32*W.T blocked
    ATd = din("AT", [128, 8 * 96], fp8)          # 32*A.T q/k/v col groups of 32
    AoTd = din("AoT", [128, 8 * 32], fp8)        # 32*Ao.T (zero-padded to 32)
    BALLd = din("BALL", [128, E], fp8)           # rows 0:9 q, 32:41 k, 64:73 v
    BoTd = din("BoT", [9, E], fp8)
    EVTd = din("EVT", [128, 4 * 32], f32)        # cols 0:16 = E (num), 16:32 = E (den)
    rzd = din("rz", [128, 1], f32)               # rezero/1024
    out_d = nc.dram_tensor("out", [128, 8 * TQ], f32, kind="ExternalOutput").ap()
    DBG = os.environ.get("KDBG")
    if DBG:
        dQ = nc.dram_tensor("dQ", [128, 8 * TQ], fp8, kind="ExternalOutput").ap()
        dK = nc.dram_tensor("dK", [128, 8 * TQ], fp8, kind="ExternalOutput").ap()
        dV = nc.dram_tensor("dV", [128, 4 * H * 128], fp8, kind="ExternalOutput").ap()
        dOT = nc.dram_tensor("dOT", [128, 8 * TQ], fp8, kind="ExternalOutput").ap()
        dKsb = nc.dram_tensor("dKsb", [128, 16 * 128], fp8, kind="ExternalOutput").ap()
        dVg = nc.dram_tensor("dVg", [128, 4 * 16 * 128], fp8, kind="ExternalOutput").ap()
        dP = nc.dram_tensor("dP", [128, 1024], fp8, kind="ExternalOutput").ap()

    with tile.TileContext(nc) as tc:
        import contextlib
        ctx = contextlib.ExitStack()
        dram = ctx.enter_context(tc.tile_pool(name="dram", bufs=1, space="DRAM"))
        kin = dram.tile([128, 8 * TQ], fp8)
        kg = dram.tile([4 * 128, 8 * TQ], fp8)
        vin = dram.tile([128, 4 * H * 128], fp8)
        vg = dram.tile([4 * 128, 4 * H * 128], fp8)

        cpool = ctx.enter_context(tc.tile_pool(name="consts", bufs=1))
        wpool = ctx.enter_context(tc.tile_pool(name="work", bufs=1))
        ppool = ctx.enter_context(tc.tile_pool(name="ptiles", bufs=LOOKAHEAD + 2))
        spool = ctx.enter_context(tc.tile_pool(name="small", bufs=2))
        psum = ctx.enter_context(tc.tile_pool(name="psum", bufs=2, space="PSUM"))

        # ---- critical-path loads on the sync HWDGE ring; the rest on the
        # scalar-engine ring ----
        x8 = wpool.tile([128, 8, TQ], fp8, name="x8")
        nc.sync.dma_start(x8[:], x8d.rearrange("p (k t) -> p k t", t=TQ))
        Ball_sb = cpool.tile([128, E], fp8, name="Ball_sb")
        nc.sync.dma_start(Ball_sb[:], BALLd[:, :])
        W_sb = {}
        for n in "kvqo":
            W_sb[n] = wpool.tile([128, 8, E], fp8, name=f"W{n}_sb")
        nc.sync.dma_start(W_sb["k"][:], Wd["k"].rearrange("p (k m) -> p k m", m=E))

        AT_sb = cpool.tile([128, 8, 96], fp8, name="AT_sb")
        nc.scalar.dma_start(AT_sb[:], ATd.rearrange("p (k m) -> p k m", m=96))
        nc.scalar.dma_start(W_sb["v"][:], Wd["v"].rearrange("p (k m) -> p k m", m=E))
        EVT_sb = cpool.tile([128, 4, 32], f32, name="EVT_sb")
        nc.scalar.dma_start(EVT_sb[:], EVTd.rearrange("p (tt c) -> p tt c", c=32))
        nc.scalar.dma_start(W_sb["q"][:], Wd["q"].rearrange("p (k m) -> p k m", m=E))

        ones1 = cpool.tile([1, 64], bf16, name="ones1")
        nc.vector.memset(ones1[:], 1.0)
        ones512 = cpool.tile([1, TQ], bf16, name="ones512")
        nc.vector.memset(ones512[:], 1.0)
        e8k = cpool.tile([1, 32], bf16, name="e8k")   # 128 at col 8
        nc.vector.memset(e8k[:], 0.0)
        nc.vector.memset(e8k[:, 8:9], 128.0)
        e8o = cpool.tile([1, 32], bf16, name="e8o")   # 256 at col 8
        nc.vector.memset(e8o[:], 0.0)
        nc.vector.memset(e8o[:, 8:9], 256.0)

        # warm the ACT exp table early (overlaps with DMAs)
        warm = cpool.tile([1, 16], f32, name="warm")
        nc.vector.memset(warm[:], 0.0)
        nc.scalar.activation(warm[:], warm[:], AF.Exp)

        # ---- t1 = lora-A down-proj for q,k,v; row groups at bases 0/32/64
        # with a trailing ones row each (e8k x ones -> 128, evict /128) ----
        t1 = wpool.tile([128, TQ], fp8, name="t1")
        for gi, c0 in ((1, 32), (0, 0), (2, 64)):   # k group first
            ps_t1 = psum.tile([32, TQ], f32, tag="big", name=f"ps_t1_{gi}")
            nc.tensor.matmul(ps_t1[:], e8k[:],
                             ones512[:], start=True, stop=False)
            for k in range(4):
                nc.tensor.matmul(ps_t1[:],
                                 AT_sb[:, 2 * k:2 * k + 2, c0:c0 + 32],
                                 x8[:, 2 * k:2 * k + 2, :],
                                 start=False, stop=(k == 3), perf_mode=DR)
            # t1 = ps/128: lora rows -> xA/4, ones row -> 1
            nc.scalar.mul(t1[32 * gi:32 * gi + 9, :],
                          ps_t1[0:9, :], 1.0 / 128.0)

        def proj_mm(ps, Wt, m, rows):
            for k in range(4):
                nc.tensor.matmul(ps[:], Wt[:, 2 * k:2 * k + 2, m * 128:(m + 1) * 128],
                                 x8[:, 2 * k:2 * k + 2, :],
                                 start=(k == 0), stop=False, perf_mode=DR)
            nc.tensor.matmul(ps[:], Ball_sb[rows, m * 128:(m + 1) * 128],
                             t1[rows, :], start=False, stop=True)

        # ---- K projection (transposed layout [d, tok]) + AllGather ----
        Kloc = wpool.tile([128, 8, TQ], fp8, name="Kloc")
        for m in range(8):
            ps = psum.tile([128, TQ], f32, tag="big", name="ps_proj")
            proj_mm(ps, W_sb["k"], m, slice(32, 41))
            nc.scalar.copy(Kloc[:, m, :], ps[:])
        nc.sync.dma_start(kin.rearrange("p (m t) -> p m t", t=TQ), Kloc[:])
        nc.gpsimd.collective_compute(
            "AllGather", mybir.AluOpType.bypass,
            replica_groups=[[0, 1, 2, 3], [4, 5, 6, 7]],
            ins=[kin.opt()], outs=[kg.opt()],
        )

        # ---- V projection (natural layout [tok, d]), E-scaled fp8,
        # + E columns for the denominators ----
        V2 = wpool.tile([128, 4, H * 128], fp8, name="V2")
        for tt in range(4):   # zero the 63-wide pad after each E column
            nc.vector.memset(
                V2[:, tt, :].rearrange("p (h c) -> p h c", c=128)[:, :, 65:128], 0.0)
        for tt in range(4):
            for nh in range(2):
                ps = psum.tile([128, 512], f32, tag="big", name="ps_projv")
                for k in range(4):
                    nc.tensor.matmul(ps[:], x8[:, 2 * k:2 * k + 2, tt * 128:(tt + 1) * 128],
                                     W_sb["v"][:, 2 * k:2 * k + 2, nh * 512:(nh + 1) * 512],
                                     start=(k == 0), stop=False, perf_mode=DR)
                nc.tensor.matmul(ps[:], t1[64:73, tt * 128:(tt + 1) * 128],
                                 Ball_sb[64:73, nh * 512:(nh + 1) * 512],
                                 start=False, stop=True)
                outv = V2[:, tt, nh * 1024:nh * 1024 + 1024]
                outv = outv.rearrange("p (n d) -> p n d", d=128)[:, :, 0:64]
                inv = ps[:].rearrange("p (n d) -> p n d", d=64)
                eap = EVT_sb[:, tt, nh * 8:(nh + 1) * 8]
                ebc = bass.AP(eap.tensor, eap.offset,
                              [list(eap.ap[0]), list(eap.ap[1]), [0, 64]])
                nc.vector.tensor_tensor(outv, inv, ebc, op=ALU.mult)
            nc.vector.tensor_copy(V2[:, tt, 64:H * 128:128], EVT_sb[:, tt, 16:32])
        nc.sync.dma_start(vin.rearrange("p (tt c) -> p tt c", c=H * 128), V2[:])
        nc.gpsimd.collective_compute(
            "AllGather", mybir.AluOpType.bypass,
            replica_groups=[[0, 1, 2, 3], [4, 5, 6, 7]],
            ins=[vin.opt()], outs=[vg.opt()],
        )

        # ---- Q projection (transposed layout [d, q]) ----
        Q_sb = wpool.tile([128, 8, TQ], fp8, name="Q_sb")
        for m in range(8):
            ps = psum.tile([128, TQ], f32, tag="big", name="ps_proj")
            proj_mm(ps, W_sb["q"], m, slice(0, 9))
            nc.scalar.copy(Q_sb[:, m, :], ps[:])

        # ---- load gathered K (per d-pair, rank-aligned key ranges) ----
        kgv = kg.rearrange("(r p) (d t) -> p r d t", p=128, t=TQ)
        Ksb = []
        for dp in range(8):
            T = TL[dp]
            t = cpool.tile([128, T * 128], fp8, name=f"Ksb{dp}")
            if T >= 4:
                nr = T // 4
                src = kgv[:, 4 - nr:4, dp, :]
                dst = t.rearrange("p (r t) -> p r t", t=512)
            else:
                tw = T * 128
                src = kgv[:, 3, dp, 512 - tw:512]
                dst = t[:]
            nc.sync.dma_start(dst, src)
            Ksb.append(t)

        # ---- load gathered V'' (per 4-tile group, needed head tail only);
        # group 3 (last keys) first: the early attention units need it ----
        vgv = vg.rearrange("(r p) (tt c) -> p r tt c", p=128, c=H * 128)
        Vg = [None] * 4
        for g in (3, 2, 1, 0):
            hmin = VG_GROUPS[g][2]
            c0 = 128 * hmin
            t = cpool.tile([128, 4, H * 128 - c0], fp8, name=f"Vg{g}")
            nc.sync.dma_start(t[:], vgv[:, g, :, c0:])
            Vg[g] = t

        def v2slice(kt, h, two):
            g, ki = kt // 4, kt % 4
            c = (h - VG_GROUPS[g][2]) * 128
            if two:
                return Vg[g][:, ki:ki + 2, c:c + 128]
            return Vg[g][:, ki, c:c + 66]

        # ---- attention, software-pipelined ----
        OT = wpool.tile([128, 8, TQ], fp8, name="OT")
        nG = len(GROUPS)
        Pt = {}
        psO = {}
        EXPSCALE = 1.0 / 8192.0    # 1/(32*32*8): descale Q'K' and /sqrt(D)

        def close_head(h):
            lsb = spool.tile([1, TQ], f32, tag="lsb", bufs=2, name=f"l{h}")
            nc.vector.tensor_copy(lsb[:], psO[h][64:65, :])
            recf = spool.tile([1, TQ], f32, tag="recf", bufs=2, name=f"rf{h}")
            nc.vector.reciprocal_approx_fast(recf[:], lsb[:])
            rec = spool.tile([1, TQ], bf16, tag="rec", bufs=2, name=f"rec{h}")
            nc.vector.tensor_copy(rec[:], recf[:])
            onum = spool.tile([64, TQ], bf16, tag="onum", bufs=2, name=f"on{h}")
            nc.vector.tensor_copy(onum[:], psO[h][0:64, :])
            bc = psum.tile([64, TQ], f32, tag="big", name=f"bc{h}")
            nc.tensor.matmul(bc[:], ones1[:], rec[:], start=True, stop=True)
            nc.vector.tensor_mul(OT[64 * (h % 2):64 * (h % 2) + 64, h // 2, :],
                                 onum[:], bc[:])
            del psO[h]

        for step in range(nG + LOOKAHEAD):
            if step < nG:
                h, kt0, paired, _, _ = GROUPS[step]
                dp, hb = h // 2, (h % 2) * 64
                koff = kt0 - (16 - TL[dp])
                ps = psum.tile([128, 1024], f32, tag="big", name=f"psS{step}")
                nc.tensor.matmul(ps[:, 0:512],
                                 Ksb[dp][hb:hb + 64, koff * 128:(koff + 1) * 128],
                                 Q_sb[hb:hb + 64, dp, :], start=True, stop=True)
                if paired:
                    nc.tensor.matmul(ps[:, 512:1024],
                                     Ksb[dp][hb:hb + 64, (koff + 1) * 128:(koff + 2) * 128],
                                     Q_sb[hb:hb + 64, dp, :], start=True, stop=True)
                P = ppool.tile([128, 1024], fp8, tag="p", name=f"P{step}")
                if paired:
                    nc.scalar.activation(P[:], ps[:], AF.Exp, scale=EXPSCALE)
                else:
                    nc.scalar.activation(P[:, 0:512], ps[:, 0:512], AF.Exp,
                                         scale=EXPSCALE)
                Pt[step] = P
                if DBG and step == len(GROUPS) - 1:
                    nc.sync.dma_start(dP, P[:])
            j = step - LOOKAHEAD
            if j < 0:
                continue
            h, kt0, paired, first, last = GROUPS[j]
            if first:
                psO[h] = psum.tile([128, TQ], f32, tag="ot", bufs=4, name=f"psO{h}")
            P = Pt.pop(j)
            if paired:
                nc.tensor.matmul(psO[h][:], v2slice(kt0, h, True),
                                 P[:].rearrange("p (two q) -> p two q", two=2),
                                 start=first, stop=last, perf_mode=DR)
            else:
                nc.tensor.matmul(psO[h][0:66, :], v2slice(kt0, h, False), P[:, 0:512],
                                 start=first, stop=last)
            if last:
                close_head(h)

        if DBG:
            nc.sync.dma_start(dQ.rearrange("p (m t) -> p m t", t=TQ), Q_sb[:])
            nc.sync.dma_start(dK.rearrange("p (m t) -> p m t", t=TQ), Kloc[:])
            nc.sync.dma_start(dV.rearrange("p (tt c) -> p tt c", c=H * 128), V2[:])
            nc.sync.dma_start(dOT.rearrange("p (m t) -> p m t", t=TQ), OT[:])
            nc.sync.dma_start(dKsb.rearrange("p (r t) -> p r t", t=512), Ksb[7].rearrange("p (r t) -> p r t", t=512))
            nc.sync.dma_start(dVg.rearrange("p (tt c) -> p tt c", c=16 * 128), Vg[3][:])

        # ---- late consts for the O path ----
        AoT_sb = cpool.tile([128, 8, 32], fp8, name="AoT_sb")
        nc.scalar.dma_start(AoT_sb[:], AoTd.rearrange("p (k m) -> p k m", m=32))
        Bo_sb = cpool.tile([9, E], fp8, name="Bo_sb")
        nc.scalar.dma_start(Bo_sb[:], BoTd[:, :])
        nc.scalar.dma_start(W_sb["o"][:], Wd["o"].rearrange("p (k m) -> p k m", m=E))
        x_f2 = wpool.tile([128, 8, TQ], f32, name="x_f2")
        nc.scalar.dma_start(x_f2[:], xfd.rearrange("p (k t) -> p k t", t=TQ))
        rz_sb = cpool.tile([128, 1], f32, name="rz_sb")
        nc.scalar.dma_start(rz_sb[:], rzd[:, :])

        # ---- lora-o down-proj: t2 = 4*(attnout @ Ao.T), ones row = 1 ----
        ps_t2 = psum.tile([32, TQ], f32, tag="big", name="ps_t2")
        nc.tensor.matmul(ps_t2[:], e8o[:], ones512[:], start=True, stop=False)
        for k in range(4):
            nc.tensor.matmul(ps_t2[:], AoT_sb[:, 2 * k:2 * k + 2, :],
                             OT[:, 2 * k:2 * k + 2, :],
                             start=False, stop=(k == 3), perf_mode=DR)
        t2 = wpool.tile([9, TQ], fp8, name="t2")
        nc.scalar.mul(t2[:], ps_t2[0:9, :], 1.0 / 256.0)

        # ---- O projection + rezero residual ----
        out_sb = wpool.tile([128, 8, TQ], f32, name="out_sb")
        for m in range(8):
            ps = psum.tile([128, TQ], f32, tag="big", name="ps_proj")
            for k in range(4):
                nc.tensor.matmul(ps[:], W_sb["o"][:, 2 * k:2 * k + 2, m * 128:(m + 1) * 128],
                                 OT[:, 2 * k:2 * k + 2, :],
                                 start=(k == 0), stop=False, perf_mode=DR)
            nc.tensor.matmul(ps[:], Bo_sb[:, m * 128:(m + 1) * 128],
                             t2[:], start=False, stop=True)
            nc.vector.scalar_tensor_tensor(
                out_sb[:, m, :], ps[:], rz_sb[:, 0:1], x_f2[:, m, :],
                op0=ALU.mult, op1=ALU.add)
        odv = out_d.rearrange("p (m t) -> p m t", t=TQ)
        nc.sync.dma_start(odv[:, 0:4, :], out_sb[:, 0:4, :])
        nc.sync.dma_start(odv[:, 4:8, :], out_sb[:, 4:8, :])
        ctx.close()

    if not os.environ.get("BASS_SKIP_COMPILE"):
        nc.compile()
    return nc


def _get_built():
    global _BUILT
    with _LOCK:
        if _BUILT is None:
            _BUILT = _build()
    return _BUILT


def _blk(a):
    """[E, X] -> [128, 8*X] contiguous, row p holds blocks k at p = e%128."""
    Ei, X = a.shape
    return np.ascontiguousarray(
        a.reshape(8, 128, X).transpose(1, 0, 2).reshape(128, 8 * X))


def _f8(a):
    return np.ascontiguousarray(
        np.clip(np.asarray(a, F32), -240, 240).astype(F8))


def _prep_inputs(inputs):
    """Host-side sharding + weight relayout. Returns in_maps for 8 cores."""
    x = np.asarray(inputs["x"], F32)
    rez = float(np.asarray(inputs["rezero"]).reshape(-1)[0])

    Wb = {n: _f8(_blk(32.0 * np.asarray(inputs["W" + n], F32).T))
          for n in "qkvo"}
    AT = np.zeros((E, 96), F32)
    AT[:, 0:8] = 32.0 * np.asarray(inputs["Aq"], F32).T
    AT[:, 32:40] = 32.0 * np.asarray(inputs["Ak"], F32).T
    AT[:, 64:72] = 32.0 * np.asarray(inputs["Av"], F32).T
    ATb = _f8(_blk(AT))
    Ao32 = np.zeros((E, 32), F32)
    Ao32[:, 0:8] = 32.0 * np.asarray(inputs["Ao"], F32).T
    AoTb = _f8(_blk(Ao32))

    BALL = np.zeros((128, E), F32)
    for rbase, n, bscale in ((0, "q", 32.0), (32, "k", 32.0), (64, "v", 32.0)):
        BALL[rbase:rbase + 8] = 16.0 * np.asarray(inputs["B" + n], F32).T
        BALL[rbase + 8] = bscale * np.asarray(inputs["b" + n], F32)
    BALLb = _f8(BALL)
    BoT = np.zeros((9, E), F32)
    BoT[0:8] = 32.0 * np.asarray(inputs["Bo"], F32).T
    BoT[8] = 1024.0 * np.asarray(inputs["bo"], F32)
    BoTb = _f8(BoT)

    slopes = 0.5 ** np.arange(H, dtype=F32)
    jpos = np.arange(S, dtype=F32)
    Efull = np.exp(slopes[:, None] * (jpos[None, :] - (S - 1))).astype(F32)  # [H,S]
    rz_vec = np.full((128, 1), rez / 1024.0, F32)

    in_maps = []
    for c in range(NC):
        b, r = c // 4, c % 4
        sl = slice(TQ * r, TQ * (r + 1))
        xT = x[b, sl, :].T                        # [E, TQ]
        # EVT [128, 4, 32]: token t = 512*r + tt*128 + p
        EVT = np.zeros((128, 4, 32), F32)
        Eloc = Efull[:, sl]                       # [H, TQ]
        for tt in range(4):
            EVT[:, tt, 0:16] = Eloc[:, tt * 128:(tt + 1) * 128].T
            EVT[:, tt, 16:32] = Eloc[:, tt * 128:(tt + 1) * 128].T
        m = {
            "x8": _f8(_blk(xT)),
            "xf": np.ascontiguousarray(_blk(xT)),
            "Wq": Wb["q"], "Wk": Wb["k"], "Wv": Wb["v"], "Wo": Wb["o"],
            "AT": ATb, "AoT": AoTb,
            "BALL": BALLb, "BoT": BoTb,
            "EVT": np.ascontiguousarray(EVT.reshape(128, 128)),
            "rz": rz_vec,
        }
        in_maps.append(m)
    return in_maps


def _unshard(res):
    out = np.zeros((B, S, E), F32)
    for c in range(NC):
        b, r = c // 4, c % 4
        o = np.asarray(res.results[c]["out"], F32)        # [128, 8*TQ]
        oT = o.reshape(128, 8, TQ).transpose(1, 0, 2).reshape(E, TQ)
        out[b, TQ * r:TQ * (r + 1), :] = oT.T
    return out


def kernel(**inputs) -> np.ndarray:
    from concourse import bass_utils

    nc = _get_built()
    in_maps = _prep_inputs(inputs)
    res = bass_utils.run_bass_kernel_spmd(nc, in_maps, core_ids=list(range(NC)))
    return _unshard(res)


if __name__ == "__main__":
    _get_built()
    print("build+compile OK")
